# revision 14
# baseline (speedup 1.0000x reference)
"""MIND loss on 8 Trainium2 NeuronCores (Bass/Tile).

Math (validated against the jax reference by a NumPy golden model, rel err
7e-5): of the 80 neighbourhood shifts only those with |tx|,|ty| <= 1 act as
real +-512-pixel shifts (affine_grid semantics); the other 72 degenerate to
blur(img2^2).  Per 128-row band only ONE of the +-512-row partners is in
bounds, so each core computes just 5 distinct response maps + Vimg:

  s0 = blur(o^2)                       weight 77 (72 + degenerate maps)
  f  = blur((o - p)^2)                 p = partner band (rows +-512)
  x  = blur((oL - oR)^2)               placed left (po) and right (mo)
  pu = blur((oL - pR)^2)  (left half), mu = blur((oR - pL)^2) (right half)
  Vimg = (blur((o1-p1)^2 + 2*o1^2 + x1^2-both-halves))/4 + eps

Separable 7-tap gaussian blur as PE matmuls: y-conv with the sq-map chunk as
the stationary operand (output lands transposed [col, row]), then x-conv with
banded weight matrices WXL/WXM/WXR against 4-chunk slabs (N=512).  Post:
b = d2 * (-1/V) (DVE), e = exp(b) (ACT, weight 77 folded as a +ln77 bias),
num-sum on GPSIMD, den via bmax (DVE max-tree) and rden = exp(-bmax), final
fused multiply+reduce (tensor_tensor_reduce) into per-column partials.
Row-crop (rows 7..1017) is resolved on the host by subtracting separately
accumulated top7/bot6 partials; column-crop by masking num edge partitions.

All per-core inputs are packed into two bf16 blobs (one DMA each) to keep
per-instruction semaphore-wait fan-in within ISA limits.
"""

import sys
import numpy as np

sys.path.insert(0, "/opt/trn_rl_repo")

import ml_dtypes  # noqa: E402

BF = ml_dtypes.bfloat16

PATCH = 7
SIGMA = 2.0
EPS = 1e-5
H = W = 1024
LN77 = float(np.log(77.0))
NORM = 80.0 * 1011.0 * 1010.0

# blob2 free-dim offsets (bf16 elements)
OFF_HA = 0
OFF_HB = 1024
OFF_HC = 2048
OFF_HD = 2560
OFF_HE = 3072
OFF_HF = 4096
OFF_WY1 = 5120
OFF_WY2 = 5248
OFF_WXL = 5376
OFF_WXM = 5504
OFF_WXR = 5632
B2_W = 5760


def _g1d():
    ax = np.arange(PATCH, dtype=np.float64) - PATCH // 2
    return (np.exp(-(ax ** 2) / (2 * SIGMA ** 2)) /
            np.sqrt(2 * np.pi * SIGMA ** 2)).astype(np.float32)


def _weight_mats():
    G = _g1d()
    WY1 = np.zeros((128, 128), np.float32)
    for t in range(128):
        for j in range(max(0, t - 3), min(128, t + 4)):
            WY1[t, j] = G[t - j + 3]
    WY2 = np.zeros((6, 128), np.float32)
    for h in range(3):
        t = h - 3
        for j in range(0, t + 4):
            WY2[h, j] = G[t - j + 3]
    for h in range(3, 6):
        t = 128 + (h - 3)
        for j in range(t - 3, 128):
            WY2[h, j] = G[t - j + 3]
    WXM = np.zeros((128, 128), np.float32)
    for i in range(128):
        for j in range(max(0, i - 3), min(128, i + 4)):
            WXM[i, j] = G[i - j + 3]
    WXL = np.zeros((128, 128), np.float32)
    for i in range(125, 128):
        for j in range(0, (i - 128) + 4):
            WXL[i, j] = G[(i - 128) - j + 3]
    WXR = np.zeros((128, 128), np.float32)
    for i in range(0, 3):
        for j in range(125 + i, 128):
            WXR[i, j] = G[(i + 128) - j + 3]
    return WY1, WY2, WXL, WXM, WXR


# ---------------------------------------------------------------- host prep

def _band(img, base):
    """rows base-3..base+130 -> [128,1024] band + [6,1024] halo, zero-padded."""
    bd = np.zeros((128, W), np.float32)
    hl = np.zeros((6, W), np.float32)
    lo, hi = max(0, base), min(H, base + 128)
    if lo < hi:
        bd[lo - base:hi - base] = img[lo:hi]
    for k in range(3):
        t = base - 3 + k
        if 0 <= t < H:
            hl[k] = img[t]
        t = base + 128 + k
        if 0 <= t < H:
            hl[3 + k] = img[t]
    return bd, hl


def _core_inputs(img1, img2, c, wblob_bf):
    r0 = c * 128
    pbase = r0 + 512 if c < 4 else r0 - 512
    o2, o2h = _band(img2, r0)
    p2, p2h = _band(img2, pbase)
    o1, o1h = _band(img1, r0)
    p1, p1h = _band(img1, pbase)

    b1 = np.concatenate([o2, p2, o1, p1], axis=1)  # [128, 4096]

    b2 = np.zeros((128, B2_W), np.float32)
    b2[0:6, OFF_HA:OFF_HA + 1024] = o2h
    b2[32:38, OFF_HA:OFF_HA + 1024] = o2h
    b2[32:38, OFF_HB:OFF_HB + 1024] = p2h
    b2[0:6, OFF_HC:OFF_HC + 512] = o2h[:, 0:512]
    b2[32:38, OFF_HC:OFF_HC + 512] = o2h[:, 0:512]
    b2[64:70, OFF_HC:OFF_HC + 512] = o2h[:, 512:]
    b2[0:6, OFF_HD:OFF_HD + 512] = o2h[:, 512:]
    b2[32:38, OFF_HD:OFF_HD + 512] = p2h[:, 512:]
    b2[64:70, OFF_HD:OFF_HD + 512] = p2h[:, 0:512]
    b2[0:6, OFF_HE:OFF_HE + 1024] = o1h
    b2[0:6, OFF_HF:OFF_HF + 1024] = p1h
    b2 = b2.astype(BF)
    b2[:, OFF_WY1:] = wblob_bf

    CM = np.ones((128, 2), np.float32)
    CM[0:7, 0] = 0.0
    CM[121:128, 1] = 0.0

    return {"blob1": b1.astype(BF), "blob2": b2, "colmask": CM}


def _make_wblob():
    WY1, WY2, WXL, WXM, WXR = _weight_mats()
    wb = np.zeros((128, B2_W - OFF_WY1), np.float32)
    wb[:, 0:128] = WY1
    wb[0:6, 128:256] = WY2
    wb[32:38, 128:256] = WY2
    wb[64:70, 128:256] = WY2
    wb[:, 256:384] = WXL
    wb[:, 384:512] = WXM
    wb[:, 512:640] = WXR
    return wb.astype(BF)


# ---------------------------------------------------------------- bass build

_NC_CACHE = {}


def _build_nc(stage=99):
    import concourse.bacc as bacc
    import concourse.mybir as mybir
    from concourse.tile import TileContext

    f32 = mybir.dt.float32
    bf16 = mybir.dt.bfloat16
    Alu = mybir.AluOpType
    Act = mybir.ActivationFunctionType

    nc = bacc.Bacc("TRN2")

    d_b1 = nc.declare_dram_parameter("blob1", [128, 4096], bf16, isOutput=False)
    d_b2 = nc.declare_dram_parameter("blob2", [128, B2_W], bf16, isOutput=False)
    d_cm = nc.declare_dram_parameter("colmask", [128, 2], f32, isOutput=False)
    out_part = nc.declare_dram_parameter("partials", [128, 3], f32, isOutput=True)

    with TileContext(nc) as tc:
        with (
            tc.tile_pool(name="pers", bufs=1) as P,
            tc.tile_pool(name="ypsum", bufs=2, space="PSUM") as YP,
            tc.tile_pool(name="xpsum", bufs=4, space="PSUM") as XP,
        ):
            _pool_body(nc, stage, P, YP, XP, d_b1, d_b2, d_cm, out_part)

    nc.finalize()
    return nc


def _pool_body(nc, stage, P, YP, XP, d_b1, d_b2, d_cm, out_part):
    import concourse.mybir as mybir
    f32 = mybir.dt.float32
    bf16 = mybir.dt.bfloat16
    Alu = mybir.AluOpType
    Act = mybir.ActivationFunctionType
    if True:
        if True:
            B1 = P.tile([128, 4096], bf16, tag="B1", name="B1")
            B2 = P.tile([128, B2_W], bf16, tag="B2", name="B2")
            colmask = P.tile([128, 2], f32, tag="colmask", name="colmask")
            nc.sync.dma_start(out=B1[:], in_=d_b1[:])
            nc.sync.dma_start(out=B2[:], in_=d_b2[:])
            nc.sync.dma_start(out=colmask[:], in_=d_cm[:])

            def v3(ap, p0, p1_, off, nch):
                return ap[p0:p1_, off:off + nch * 128].rearrange(
                    "p (a b) -> p a b", b=128)

            o2 = v3(B1, 0, 128, 0, 8)
            p2 = v3(B1, 0, 128, 1024, 8)
            o1 = v3(B1, 0, 128, 2048, 8)
            p1 = v3(B1, 0, 128, 3072, 8)
            hA = v3(B2, 0, 38, OFF_HA, 8)
            hB = v3(B2, 0, 38, OFF_HB, 8)
            hC = v3(B2, 0, 70, OFF_HC, 4)
            hD = v3(B2, 0, 70, OFF_HD, 4)
            hE = v3(B2, 0, 6, OFF_HE, 8)
            hF = v3(B2, 0, 6, OFF_HF, 8)
            wy1 = B2[:, OFF_WY1:OFF_WY1 + 128]
            wy2 = B2[:, OFF_WY2:OFF_WY2 + 128]
            wxl = B2[:, OFF_WXL:OFF_WXL + 128]
            wxm = B2[:, OFF_WXM:OFF_WXM + 128]
            wxr = B2[:, OFF_WXR:OFF_WXR + 128]

            def tile(tag, shp, dt=bf16):
                return P.tile(shp, dt, tag=tag, name=tag)

            V = nc.vector
            GP = nc.gpsimd
            SC = nc.scalar

            partials = tile("partials", [128, 3], f32)

            # ---- sq stage (main band)
            d_f = tile("d_f", [128, 8, 128])
            V.tensor_sub(d_f[:], o2, p2)
            sq0 = tile("sq0", [128, 8, 128])
            GP.tensor_mul(sq0[:], o2, o2)
            sqf = tile("sqf", [128, 8, 128])
            GP.tensor_mul(sqf[:], d_f[:], d_f[:])

            d_x = tile("d_x", [128, 4, 128])
            V.tensor_sub(d_x[:], o2[:, 0:4, :], o2[:, 4:8, :])
            sqx = tile("sqx", [128, 4, 128])
            V.tensor_mul(sqx[:], d_x[:], d_x[:])
            d_p = tile("d_p", [128, 4, 128])
            V.tensor_sub(d_p[:], o2[:, 0:4, :], p2[:, 4:8, :])
            sqp = tile("sqp", [128, 4, 128])
            V.tensor_mul(sqp[:], d_p[:], d_p[:])
            d_m = tile("d_m", [128, 4, 128])
            V.tensor_sub(d_m[:], o2[:, 4:8, :], p2[:, 0:4, :])
            sqm = tile("sqm", [128, 4, 128])
            V.tensor_mul(sqm[:], d_m[:], d_m[:])

            d_v = tile("d_v", [128, 8, 128])
            V.tensor_sub(d_v[:], o1, p1)
            v1sq = tile("v1sq", [128, 8, 128])
            GP.tensor_mul(v1sq[:], d_v[:], d_v[:])
            o1sq = tile("o1sq", [128, 8, 128])
            GP.tensor_mul(o1sq[:], o1, o1)
            sqsv = tile("sqsv", [128, 8, 128])
            V.scalar_tensor_tensor(sqsv[:], o1sq[:], 2.0, v1sq[:],
                                   Alu.mult, Alu.add)
            d_vx = tile("d_vx", [128, 4, 128])
            V.tensor_sub(d_vx[:], o1[:, 0:4, :], o1[:, 4:8, :])
            sqvx = tile("sqvx", [128, 4, 128])
            V.tensor_mul(sqvx[:], d_vx[:], d_vx[:])

            # ---- sq stage (halos, on gpsimd)
            dAB = tile("dAB", [38, 8, 128])
            GP.tensor_sub(dAB[:], hA, hB)
            sqH1 = tile("sqH1", [38, 8, 128])
            GP.tensor_mul(sqH1[:], dAB[:], dAB[:])
            dCD = tile("dCD", [70, 4, 128])
            GP.tensor_sub(dCD[:], hC, hD)
            sqH3 = tile("sqH3", [70, 4, 128])
            GP.tensor_mul(sqH3[:], dCD[:], dCD[:])
            dEF = tile("dEF", [6, 8, 128])
            GP.tensor_sub(dEF[:], hE, hF)
            v1h = tile("v1h", [6, 8, 128])
            GP.tensor_mul(v1h[:], dEF[:], dEF[:])
            o1h2 = tile("o1h2", [6, 8, 128])
            GP.tensor_mul(o1h2[:], hE, hE)
            svh = tile("svh", [6, 8, 128])
            V.scalar_tensor_tensor(svh[:], o1h2[:], 2.0, v1h[:],
                                   Alu.mult, Alu.add)
            d_vxh = tile("d_vxh", [6, 4, 128])
            GP.tensor_sub(d_vxh[:], hE[:, 0:4, :], hE[:, 4:8, :])
            vxh = tile("vxh", [6, 4, 128])
            GP.tensor_mul(vxh[:], d_vxh[:], d_vxh[:])

            if stage < 2:
                V.memset(partials[:], 1.0)
                nc.sync.dma_start(out=out_part[:], in_=partials[:])
                return

            # ---- s1 tiles (y-conv, transposed [col, row]); slot k = chunk k-1
            s1_0 = tile("s1_0", [128, 10, 128])
            s1_f = tile("s1_f", [128, 10, 128])
            s1_V = tile("s1_V", [128, 10, 128])
            s1_x = tile("s1_x", [128, 6, 128])
            s1_p = tile("s1_p", [128, 6, 128])
            s1_m = tile("s1_m", [128, 6, 128])
            for t10 in (s1_0, s1_f, s1_V):
                V.memset(t10[:, 0, :], 0.0)
                V.memset(t10[:, 9, :], 0.0)
            for t6 in (s1_x, s1_p, s1_m):
                V.memset(t6[:, 0, :], 0.0)
                V.memset(t6[:, 5, :], 0.0)

            MM = nc.tensor.matmul

            copy_flip = [0]

            def psum_to_sbuf(dst_ap, src_ap):
                # alternate the PSUM->SBUF copies between ACT and DVE
                if copy_flip[0] % 2 == 0:
                    SC.copy(dst_ap, src_ap)
                else:
                    V.tensor_copy(dst_ap, src_ap)
                copy_flip[0] += 1

            # ---- Vimg y-stage (4 accumulating MMs per chunk), then to SBUF
            for bank in range(2):
                yp = YP.tile([128, 4, 128], f32, tag="yp", name="yp")
                for k in range(4):
                    c = 4 * bank + k
                    fc = c % 4
                    MM(yp[:, k, :], sqsv[:, c, :], wy1, start=True, stop=False)
                    MM(yp[:, k, :], svh[:, c, :], wy2[0:6, :],
                       start=False, stop=False)
                    MM(yp[:, k, :], sqvx[:, fc, :], wy1, start=False, stop=False)
                    MM(yp[:, k, :], vxh[:, fc, :], wy2[0:6, :],
                       start=False, stop=True)
                psum_to_sbuf(s1_V[:, 1 + 4 * bank:5 + 4 * bank, :], yp[:])

            # ---- main y-stage
            ygroups = [
                (sq0, sqH1, 0, s1_0, 8),
                (sqf, sqH1, 32, s1_f, 8),
                (sqx, sqH3, 0, s1_x, 4),
                (sqp, sqH3, 32, s1_p, 4),
                (sqm, sqH3, 64, s1_m, 4),
            ]
            for sq, hsq, hbase, dst, nch in ygroups:
                for bank in range(nch // 4):
                    yp = YP.tile([128, 4, 128], f32, tag="yp", name="yp")
                    for k in range(4):
                        c = 4 * bank + k
                        MM(yp[:, k, :], sq[:, c, :], wy1, start=True, stop=False)
                        MM(yp[:, k, :], hsq[hbase:hbase + 6, c, :],
                           wy2[hbase:hbase + 6, :], start=False, stop=True)
                    psum_to_sbuf(dst[:, 1 + 4 * bank:5 + 4 * bank, :], yp[:])

            if stage < 3:
                V.memset(partials[:], 1.0)
                nc.sync.dma_start(out=out_part[:], in_=partials[:])
                return

            # ---- x-stage slabs (PSUM [128, 4 chunks, 128 rows])
            def xslab(s1t, base, fixes=()):
                xp = XP.tile([128, 4, 128], f32, tag="xp", name="xp")
                MM(xp[:], wxm, s1t[:, base + 1:base + 5, :],
                   start=True, stop=False)
                for wfix, rhs_fix, blk in fixes:
                    MM(xp[:, blk, :], wfix, rhs_fix, start=False, stop=False)
                MM(xp[:], wxl, s1t[:, base + 0:base + 4, :],
                   start=False, stop=False)
                MM(xp[:], wxr, s1t[:, base + 2:base + 6, :],
                   start=False, stop=True)
                return xp

            # Vimg first so vinv is ready early
            vL = xslab(s1_V, 0)
            vR = xslab(s1_V, 4)

            nv = tile("nv", [128, 8, 128], f32)
            nvinv = tile("nvinv", [128, 8, 128], f32)
            V.tensor_scalar(nv[:, 0:4, :], vL[:], -0.25, -EPS,
                            Alu.mult, Alu.add)
            V.reciprocal_approx_fast(out=nvinv[:, 0:4, :], in_=nv[:, 0:4, :])
            V.tensor_scalar(nv[:, 4:8, :], vR[:], -0.25, -EPS,
                            Alu.mult, Alu.add)
            V.reciprocal_approx_fast(out=nvinv[:, 4:8, :], in_=nv[:, 4:8, :])

            if stage < 4:
                V.memset(partials[:], 1.0)
                nc.sync.dma_start(out=out_part[:], in_=partials[:])
                return

            slabs = [
                ("s0L", xslab(s1_0, 0), 0),
                ("fL", xslab(s1_f, 0), 0),
                ("po", xslab(s1_x, 0, fixes=((wxr, s1_0[:, 5, :], 3),)), 0),
                ("pu", xslab(s1_p, 0, fixes=((wxr, s1_0[:, 5, :], 3),)), 0),
                ("s0R", xslab(s1_0, 4), 1),
                ("fR", xslab(s1_f, 4), 1),
                ("mo", xslab(s1_x, 0, fixes=((wxl, s1_0[:, 4, :], 0),)), 1),
                ("mu", xslab(s1_m, 0, fixes=((wxl, s1_0[:, 4, :], 0),)), 1),
            ]

            # ---- post stage
            ln77 = tile("ln77", [128, 1], f32)
            V.memset(ln77[:], LN77)
            A = [tile("A_L", [128, 4, 4, 128]), tile("A_R", [128, 4, 4, 128])]
            E = [tile("E_L", [128, 4, 4, 128]), tile("E_R", [128, 4, 4, 128])]
            num = tile("num", [128, 8, 128])
            bmax = tile("bmax", [128, 8, 128])
            rden = tile("rden", [128, 8, 128])
            tmax = tile("tmax", [128, 2, 4, 128])
            tnum = tile("tnum", [128, 2, 4, 128])

            for name, xp, half in slabs:
                m = {"s0L": 0, "s0R": 0, "fL": 1, "fR": 1,
                     "po": 2, "mo": 2, "pu": 3, "mu": 3}[name]
                nvs = nvinv[:, 4 * half:4 * half + 4, :]
                V.tensor_tensor(A[half][:, m], xp[:], nvs, Alu.mult)

            if stage < 5:
                V.memset(partials[:], 1.0)
                nc.sync.dma_start(out=out_part[:], in_=partials[:])
                return

            for half in range(2):
                Ah, Eh = A[half], E[half]
                cs = slice(4 * half, 4 * half + 4)
                if stage >= 5.5:
                    SC.activation(Eh[:, 0], Ah[:, 0], Act.Exp, bias=ln77[:])
                    SC.activation(Eh[:, 1:4], Ah[:, 1:4], Act.Exp)
                V.tensor_tensor(tmax[:, 0], Ah[:, 0], Ah[:, 1], Alu.max)
                V.tensor_tensor(tmax[:, 1], Ah[:, 2], Ah[:, 3], Alu.max)
                V.tensor_tensor(bmax[:, cs, :], tmax[:, 0], tmax[:, 1], Alu.max)
                if stage >= 6:
                    GP.tensor_add(tnum[:, 0], Eh[:, 0], Eh[:, 1])
                    GP.tensor_add(tnum[:, 1], Eh[:, 2], Eh[:, 3])
                    GP.tensor_add(num[:, cs, :], tnum[:, 0], tnum[:, 1])

            if stage >= 6.5:
                SC.activation(rden[:], bmax[:], Act.Exp, scale=-1.0)

            if stage < 7:
                V.memset(partials[:], 1.0)
                nc.sync.dma_start(out=out_part[:], in_=partials[:])
                return

            # column crop: zero global cols 0-6 and 1017-1023 in num
            V.tensor_scalar(num[:, 0, :], num[:, 0, :], colmask[:, 0:1], None,
                            Alu.mult)
            V.tensor_scalar(num[:, 7, :], num[:, 7, :], colmask[:, 1:2], None,
                            Alu.mult)

            if stage < 7.4:
                V.memset(partials[:], 1.0)
                nc.sync.dma_start(out=out_part[:], in_=partials[:])
                return

            # ---- final fused multiply + reduce to per-column partials
            scr = tile("scr", [128, 8, 128])
            V.scalar_tensor_tensor(
                scr[:], num[:], 1.0, rden[:], Alu.mult, Alu.mult,
                accum_out=partials[:, 0:1])
            V.scalar_tensor_tensor(
                scr[:, :, 0:7], num[:, :, 0:7], 1.0, rden[:, :, 0:7],
                Alu.mult, Alu.mult, accum_out=partials[:, 1:2])
            V.scalar_tensor_tensor(
                scr[:, :, 122:128], num[:, :, 122:128], 1.0,
                rden[:, :, 122:128], Alu.mult, Alu.mult,
                accum_out=partials[:, 2:3])

            nc.sync.dma_start(out=out_part[:], in_=partials[:])


def _get_nc():
    if "nc" not in _NC_CACHE:
        _NC_CACHE["nc"] = _build_nc()
    return _NC_CACHE["nc"]


# ---------------------------------------------------------------- entry point

def kernel(image1, image2, _trace=False):
    from concourse.bass_utils import run_bass_kernel_spmd

    img1 = np.asarray(image1, np.float32)[0, 0]
    img2 = np.asarray(image2, np.float32)[0, 0]
    wblob = _make_wblob()
    in_maps = [_core_inputs(img1, img2, c, wblob) for c in range(8)]

    nc = _get_nc()
    res = run_bass_kernel_spmd(nc, in_maps, list(range(8)), trace=_trace)
    if _trace:
        print("exec_time_ns:", res.exec_time_ns)
        _NC_CACHE["last_exec_ns"] = res.exec_time_ns

    total = 0.0
    for c, r in enumerate(res.results):
        p = np.asarray(r["partials"], np.float64)
        s = p[:, 0].sum()
        if c == 0:
            s -= p[:, 1].sum()
        if c == 7:
            s -= p[:, 2].sum()
        total += s
    return np.float32(total / NORM)
